# revision 40
# baseline (speedup 1.0000x reference)
"""AttentionFuserV3 Trainium2 kernel: 8-core pure data parallel over batch.

v2: fp8 DoubleRow on the probability/output matmuls of stage 1.

Reference computation per batch item x_b [L=1024, D=512]:
  stage1: q = x W1^T; S = q x^T; A = softmax(S); mix = A x;
          h = tanh([mix, q] Wo1^T); h = h / max(||h||_2, eps)     (per row)
  stage2: c = [h, x]; q2 = c W2^T; S2 = q2 c^T; A2 = softmax(S2);
          mix2 = A2 c; o = [mix2, q2] Wo2^T; emb = mean_l(o)

Layout: T-space for stage 1 (features on partitions), natural orientation
for stage-2 attention (see ph8).  Stage-2 exploits linearity of the final
mean (r-trick): emb = (1/L)[c^T r ; sum_l q2] Wo2^T.

fp8 (e4m3) with perf_mode=DoubleRow (K=256 per instruction, ~2x PE
throughput at N=512) is used where measured end-to-end error stays small:
  - ph3 (mix = A x): probs are normalized to [0,1] on DVE (exp_bf16 x
    recip -> fp8) and x ships as fp8 from the host.  (3.1e-3 alone)
  - ph4 (out1 = [mix,q] Wo1^T): Wo1 ships as fp8 x64 (tanh applies
    scale=1/64), mix/q are fp8 copies of their psums.  (4.3e-3 alone)
Scores, W1/W2 projections and stage 2 stay f32r: fp8 there measured
1e-2..8e-2 end-to-end (softmax logit noise) -- over the 2e-2 gate.

DVE relief: the stage-2 softmax denominators (dsum) and the q2 column
sums (q2red) ride the ACT accumulator (accum_out) of the exp /
psum-copy instructions instead of DVE tensor_reduces.
"""

import sys

sys.path.insert(0, "/opt/trn_rl_repo")

import numpy as np

N_GLOBAL, L, D = 32, 1024, 512
NCORES = 8
B = N_GLOBAL // NCORES          # 4 batch items per core
P = 128
LC = 512                        # l-chunk (matmul moving free dim)
NLC = L // LC                   # 2
DT = D // P                     # 4
LT = L // P                     # 8
D2T = 2 * D // P                # 8
C2T = 4 * D // P                # 16

_CACHE = {}


def _build_nc(xt_bufs=2):
    import concourse.bass as bass  # noqa: F401
    import concourse.mybir as mybir
    import concourse.tile as tile
    from concourse import bacc

    f32 = mybir.dt.float32
    f32r = mybir.dt.float32r
    bf16 = mybir.dt.bfloat16
    fp8 = mybir.dt.float8e4
    AF = mybir.ActivationFunctionType
    ALU = mybir.AluOpType
    AXX = mybir.AxisListType.X
    DR = mybir.MatmulPerfMode.DoubleRow

    nc = bacc.Bacc("TRN2", target_bir_lowering=False, debug=False,
                   num_devices=NCORES)

    x_ext = nc.declare_dram_parameter("x", [B, L, D], bf16, isOutput=False)
    x8_ext = nc.declare_dram_parameter("x8", [B, L, D], fp8, isOutput=False)
    xT_ext = nc.declare_dram_parameter("xT", [B, D, L], f32r, isOutput=False)
    w1t_ext = nc.declare_dram_parameter("w1t", [D, D], f32r, isOutput=False)
    wo1t8_ext = nc.declare_dram_parameter("wo1t8", [2 * D, D], fp8, isOutput=False)
    w2t_ext = nc.declare_dram_parameter("w2t", [2 * D, 2 * D], f32r, isOutput=False)
    wo2t_ext = nc.declare_dram_parameter("wo2t", [4 * D, D], bf16, isOutput=False)
    # Constants shipped from host: walrus's ISA check rejects memset/iota
    # writes into float32r tiles, but DMA from an f32r DRAM param is fine.
    onm_ext = nc.declare_dram_parameter("onesm", [P, P], bf16, isOutput=False)
    onr_ext = nc.declare_dram_parameter("onesr", [1, P], bf16, isOutput=False)
    out_ext = nc.declare_dram_parameter("out", [B, D], f32, isOutput=True)

    import time as _time
    _t0 = _time.time()
    with tile.TileContext(nc) as tc:
        with tc.tile_pool(name="wp", bufs=1) as wp, \
             tc.tile_pool(name="cp", bufs=1) as cp, \
             tc.tile_pool(name="xtp", bufs=xt_bufs) as xtp, \
             tc.tile_pool(name="xp", bufs=2) as xp, \
             tc.tile_pool(name="hp", bufs=2) as hp, \
             tc.tile_pool(name="tp", bufs=1) as tp, \
             tc.tile_pool(name="vp", bufs=2) as vp, \
             tc.tile_pool(name="ep", bufs=1) as ep, \
             tc.tile_pool(name="ps", bufs=8, space="PSUM") as pp:

            # ---- w1t gates ph1 of item 0: DMA it first (chunks of it
            # interleave with the xT chunks inside the b==0 iteration)
            w1t_s = wp.tile([P, DT, D], f32r, tag="w1t")
            onesm_s = cp.tile([P, P], bf16, tag="onesm")
            nc.sync.dma_start(out=onesm_s, in_=onm_ext[:, :])
            onesr_s = cp.tile([1, P], bf16, tag="onesr")
            nc.sync.dma_start(out=onesr_s, in_=onr_ext[:, :])

            wo1t8_s = wp.tile([P, D2T, D], fp8, tag="wo1t8")
            w2t_s = wp.tile([P, D2T, 2 * D], f32r, tag="w2t")
            wo2t_s = wp.tile([P, C2T, D], bf16, tag="wo2t")

            # meanvec columns for the deferred Wo2 projection:
            # c-chunks 0..3 = sum_l mix2 (h part), 4..7 (x part), 8..15 = sum_l q2
            mv_s = ep.tile([P, C2T, B], bf16, tag="mv")

            def mm(out, lhsT, rhs, first, last, pm=None):
                nc.tensor.matmul(out, lhsT, rhs, start=first, stop=last,
                                 perf_mode=pm)

            def bc_recip(denom_ps, b, lc):
                """[128,512] PSUM pre-broadcast softmax denominator ->
                f32 + bf16 approx reciprocals."""
                bc = vp.tile([P, LC], f32, tag="bc", bufs=2,
                             name=f"bc_{b}_{lc}")
                bcb = vp.tile([P, LC], bf16, tag="bcb", bufs=1,
                              name=f"bcb_{b}_{lc}")
                nc.vector.reciprocal_approx_fast(out=bc, in_=denom_ps)
                with nc.allow_low_precision(reason="bf16 softmax scale"):
                    nc.vector.tensor_copy(bcb, bc)
                return bcb

            def emit_normA8(expT_s, bcb, b, lc):
                """Normalized attention probs in fp8: A8 = expT * (1/denom)."""
                A8_s = tp.tile([P, LT, LC], fp8, tag="a8", bufs=2,
                               name=f"a8_{b}_{lc}")
                with nc.allow_low_precision(reason="fp8 probs"):
                    for mt in range(LT):
                        nc.vector.tensor_mul(A8_s[:, mt, :], expT_s[:, mt, :],
                                             bcb)
                return A8_s

            def emit_ph1(xT_s, b, lc):
                """q chunks: f32r copy (ph2 rhs) + fp8 copy (ph4 rhs)."""
                ls = slice(lc * LC, (lc + 1) * LC)
                qT_s = tp.tile([P, DT, LC], f32r, tag="qt", bufs=1,
                               name=f"qt_{b}_{lc}")
                q8T_s = tp.tile([P, DT, LC], fp8, tag="q8", bufs=2,
                                name=f"q8_{b}_{lc}")
                pss = []
                for et in range(DT):
                    ps = pp.tile([P, LC], f32, tag="ps")
                    for dk in range(DT):
                        mm(ps, w1t_s[:, dk, et * P:(et + 1) * P],
                           xT_s[:, dk, ls], dk == 0, dk == DT - 1)
                    # qT alternates ACT/DVE (ph2 needs it soon); q8 copies
                    # all ride DVE afterwards -- ACT must stay clear for the
                    # next ph2's exps (the deferred denominator matmuls
                    # stall the PE on them)
                    with nc.allow_low_precision(reason="f32r stores"):
                        if et % 2 == 0:
                            nc.scalar.copy(qT_s[:, et, :], ps)
                        else:
                            nc.vector.tensor_copy(qT_s[:, et, :], ps)
                    pss.append(ps)
                with nc.allow_low_precision(reason="fp8 stores"):
                    for et in range(DT):
                        nc.vector.tensor_copy(q8T_s[:, et, :], pss[et])
                return qT_s, q8T_s

            def emit_ph2(xT_s, qT_s, b, lc):
                expT_s = tp.tile([P, LT, LC], bf16, tag="exp",
                                 name=f"exp1_{b}_{lc}")
                ps_d = pp.tile([P, LC], f32, tag="ps")
                # denominator accumulation deferred TWO mt so each
                # ones-matmul hides two tiles behind its exp: the ACT queue
                # (copies ahead of the exps) gets ~2us of slack before the
                # PE stream would stall on a denominator's exp
                for mt in range(LT):
                    ps = pp.tile([P, LC], f32, tag="ps")
                    for ek in range(DT):
                        mm(ps, xT_s[:, ek, mt * P:(mt + 1) * P],
                           qT_s[:, ek, :], ek == 0, ek == DT - 1)
                    if mt > 1:
                        mm(ps_d, onesm_s, expT_s[:, mt - 2, :],
                           mt - 2 == 0, False)
                    nc.scalar.activation(expT_s[:, mt, :], ps, AF.Exp)
                mm(ps_d, onesm_s, expT_s[:, LT - 2, :], False, False)
                mm(ps_d, onesm_s, expT_s[:, LT - 1, :], False, True)
                return expT_s, ps_d

            def emit_ph3_dr(x8_s, A8_s, b, lc):
                """mix = A x as fp8 DoubleRow over m-chunk pairs; A8 is
                normalized so the psum is the final mix -> fp8 copy."""
                mix8_s = tp.tile([P, DT, LC], fp8, tag="mix8", bufs=1,
                                 name=f"mix8_{b}_{lc}")
                for dt in range(DT):
                    ps = pp.tile([P, LC], f32, tag="ps")
                    for mk in range(0, LT, 2):
                        mm(ps, x8_s[:, mk:mk + 2, dt * P:(dt + 1) * P],
                           A8_s[:, mk:mk + 2, :], mk == 0, mk == LT - 2,
                           pm=DR)
                    with nc.allow_low_precision(reason="fp8 mix store"):
                        nc.scalar.copy(mix8_s[:, dt, :], ps)
                return mix8_s

            def emit_ph4_dr(mix8_s, q8T_s, hTn_s, lc):
                """out1 -> tanh(psum/64) via fp8 DoubleRow over c-chunk
                pairs (Wo1 ships as fp8 x64)."""
                ls = slice(lc * LC, (lc + 1) * LC)
                for ot in range(DT):
                    ps = pp.tile([P, LC], f32, tag="ps")
                    for ck in range(0, D2T, 2):
                        rhs = (mix8_s[:, ck:ck + 2, :] if ck < DT
                               else q8T_s[:, ck - DT:ck - DT + 2, :])
                        mm(ps, wo1t8_s[:, ck:ck + 2, ot * P:(ot + 1) * P],
                           rhs, ck == 0, ck == D2T - 2, pm=DR)
                    nc.scalar.activation(hTn_s[:, ot, ls], ps, AF.Tanh,
                                         scale=1.0 / 64)

            def emit_ph5(hTn_s, b, lc):
                """L2-normalize hTn[:, :, ls] in place (norm over the
                partition axis via ones-matmul)."""
                ls = slice(lc * LC, (lc + 1) * LC)
                # hsq = 16*h^2 in fp8 (the x16 keeps h^2 out of e4m3's
                # subnormal range, where flushing biases the norm low);
                # the sqrt's scale=1/16 undoes it exactly
                hsq_s = tp.tile([P, DT, LC], fp8, tag="hsq", name=f"hsq_{b}_{lc}")
                with nc.allow_low_precision(reason="fp8 16*h^2 for norm"):
                    for dt in range(DT):
                        nc.vector.scalar_tensor_tensor(
                            hsq_s[:, dt, :], hTn_s[:, dt, ls], 16.0,
                            hTn_s[:, dt, ls], op0=ALU.mult, op1=ALU.mult)
                ps_n = pp.tile([P, LC], f32, tag="ps")
                for dt in range(DT):
                    mm(ps_n, onesm_s, hsq_s[:, dt, :], dt == 0, dt == DT - 1)
                bcn = vp.tile([P, LC], f32, tag="bc", bufs=2, name=f"bcn_{b}_{lc}")
                bc2 = vp.tile([P, LC], f32, tag="bc", bufs=2, name=f"bc2_{b}_{lc}")
                nc.scalar.activation(bcn, ps_n, AF.Sqrt, scale=1.0 / 16)
                nc.vector.tensor_scalar_max(bcn, bcn, 1e-12)
                nc.vector.reciprocal_approx_fast(out=bc2, in_=bcn)
                for dt in range(DT):
                    nc.vector.tensor_mul(hTn_s[:, dt, ls], hTn_s[:, dt, ls], bc2)

            def emit_ph7(hTn_s, xT_s, q2red_s, q2T_s, lc, et_lo, et_hi):
                ls = slice(lc * LC, (lc + 1) * LC)

                def c2T(k, fs):
                    return hTn_s[:, k, fs] if k < DT else xT_s[:, k - DT, fs]

                for et in range(et_lo, et_hi):
                    ps = pp.tile([P, LC], f32, tag="ps")
                    for dk in range(D2T):
                        mm(ps, w2t_s[:, dk, et * P:(et + 1) * P],
                           c2T(dk, ls), dk == 0, dk == D2T - 1)
                    # psum->SBUF copy accumulates the q2 column-sum partial
                    # on the ACT accumulator: a DVE tensor_reduce here would
                    # read f32r (2x slow path, ~1.1us) and clog the DVE FIFO
                    # ahead of ph8's u-chain
                    nc.scalar.activation(q2T_s[:, et, :], ps, AF.Copy,
                                         accum_out=q2red_s[:, et, lc:lc + 1])

            def emit_ph8_nat(hTn_s, xT_s, q2T_s, rrow_ps, b, lc):
                """Stage-2 attention in NATURAL orientation (query l on
                partitions): per l-tile, the softmax denominator rides the
                exp's ACT accumulator and r accumulates via matmuls with the
                reciprocal vector as lhsT -- r = sum_lt u_lt^T @ exp2n_lt."""
                def c2T(k, fs):
                    return hTn_s[:, k, fs] if k < DT else xT_s[:, k - DT, fs]

                pend_u = [None]

                def flush_u():
                    if pend_u[0] is not None:
                        pu_b, pe2n, plt = pend_u[0]
                        pend_u[0] = None
                        for ms in range(NLC):
                            mm(rrow_ps[ms][0:1, :], pu_b, pe2n[:, ms, :],
                               plt == 0, plt == LT - 1)

                for li in range(LT // NLC):
                    lt = lc * (LT // NLC) + li
                    loff = li * P
                    e2n_s = tp.tile([P, NLC, LC], bf16, tag="e2n",
                                    name=f"e2n_{b}_{lt}", bufs=2)
                    dsum = vp.tile([P, 3], f32, tag="dsum", bufs=3,
                                   name=f"dsum_{b}_{lt}")
                    ps2 = [pp.tile([P, LC], f32, tag="ps",
                                   name=f"ps8_{b}_{lt}_{i}") for i in range(NLC)]
                    for ek in range(D2T):
                        for ms in range(NLC):
                            mm(ps2[ms], q2T_s[:, ek, loff:loff + P],
                               c2T(ek, slice(ms * LC, (ms + 1) * LC)),
                               ek == 0, ek == D2T - 1)
                    flush_u()   # previous lt's u-matmuls, now chain-covered
                    for ms in range(NLC):
                        nc.scalar.activation(e2n_s[:, ms, :], ps2[ms], AF.Exp,
                                             accum_out=dsum[:, ms:ms + 1])
                    nc.vector.tensor_reduce(dsum[:, 2:3], dsum[:, 0:2],
                                            axis=AXX, op=ALU.add)
                    u_f = vp.tile([P, 1], f32, tag="uf", bufs=3,
                                  name=f"uf_{b}_{lt}")
                    u_b = vp.tile([P, 1], bf16, tag="ub", bufs=3,
                                  name=f"ub_{b}_{lt}")
                    nc.vector.reciprocal_approx_fast(out=u_f, in_=dsum[:, 2:3])
                    with nc.allow_low_precision(reason="bf16 softmax scale"):
                        nc.vector.tensor_copy(u_b, u_f)
                    pend_u[0] = (u_b, e2n_s, lt)
                return flush_u

            def make_tail(b, hTn_s, xT_s, x_s, rrow_ps, q2red_s,
                          do_q2mv=True, fink=None):
                """Item tail, split in three so it can be emitted interleaved
                into the next item's stage-1 engine streams."""
                st = {}

                def tail_a():
                    if do_q2mv:
                        with nc.allow_low_precision(reason="f32r sums"):
                            nc.vector.tensor_reduce(mv_s[:, D2T:C2T, b:b + 1],
                                                    q2red_s, axis=AXX,
                                                    op=ALU.add)
                    rflat_s = vp.tile([1, L], bf16, tag="rflat", bufs=1,
                                      name=f"rflat_{b}")
                    nc.scalar.copy(rflat_s[0:1, 0:LC], rrow_ps[0][0:1, :])
                    nc.scalar.copy(rflat_s[0:1, LC:L], rrow_ps[1][0:1, :])
                    # r row -> column chunks: K=1 matmuls into disjoint
                    # columns of one psum bank
                    rc_ps = pp.tile([P, LT], f32, tag="ps", name=f"rc_{b}")
                    for mt in range(LT):
                        mm(rc_ps[:, mt:mt + 1],
                           rflat_s[0:1, mt * P:(mt + 1) * P],
                           onesr_s[0:1, 0:1], mt == 0, mt == LT - 1)
                    rsum_s = vp.tile([P, LT], bf16, tag="rsum", bufs=1,
                                     name=f"rsum_{b}")
                    with nc.allow_low_precision(reason="bf16 r"):
                        nc.vector.tensor_copy(rsum_s, rc_ps)
                    st["rflat"] = rflat_s
                    st["rsum"] = rsum_s

                def tail_b():
                    rbc_s = vp.tile([P, L], bf16, tag="rbc", bufs=1,
                                    name=f"rbc_{b}")
                    for j in range(NLC):
                        ps_b = pp.tile([P, LC], f32, tag="ps")
                        mm(ps_b, onesr_s, st["rflat"][0:1, j * LC:(j + 1) * LC],
                           True, True)
                        nc.scalar.copy(rbc_s[:, j * LC:(j + 1) * LC], ps_b)
                    st["rbc"] = rbc_s

                def tail_cx():
                    rsum_s = st["rsum"]
                    with nc.allow_low_precision(reason="f32r rounding of sums"):
                        # x part: sum_m x[m,d] r[m] as tiny bf16 matmuls
                        for dt in range(DT):
                            ps_x = pp.tile([P, 1], f32, tag="ps",
                                           name=f"psx_{b}_{dt}")
                            for mk in range(LT):
                                mm(ps_x, x_s[:, mk, dt * P:(dt + 1) * P],
                                   rsum_s[:, mk:mk + 1], mk == 0, mk == LT - 1)
                            nc.vector.tensor_copy(mv_s[:, DT + dt, b:b + 1],
                                                  ps_x)
                            if fink:
                                fink(DT + dt, False)

                def tail_ch(dt):
                    # h part: one weighted row-sum per call -- multiply on
                    # GpSimd (pure slack) into a bf16 scratch, reduce the
                    # bf16 on DVE (f32r reduces hit a 2x slow path and the
                    # in-place f32r write chained Pool behind DVE).
                    # Scratch rides the hsq tag: hsq lives in stage 1, tscr
                    # in stage 2 -- strictly sequential lifetimes.
                    scr = tp.tile([P, L], bf16, tag="hsq", bufs=1,
                                  name=f"tscr_{b}_{dt}")
                    with nc.allow_low_precision(reason="bf16 tail products"):
                        nc.gpsimd.tensor_mul(scr, hTn_s[:, dt, :], st["rbc"])
                        nc.vector.tensor_reduce(mv_s[:, dt, b:b + 1],
                                                scr, axis=AXX, op=ALU.add)
                    if fink:
                        fink(dt, dt == DT - 1)

                def tail_final():
                    # end-of-kernel drain: nothing covers this, so run the
                    # four multiplies two-per-engine first, then the reduces
                    # (+ the final Wo2 matmuls via fink) in order
                    scrs = []
                    # one scratch per dead stage-1 tag so all four products
                    # can be outstanding at once
                    scr_tags = [("hsq", 1), ("exp", 1), ("a8", 2), ("mix8", 1)]
                    with nc.allow_low_precision(reason="bf16 tail products"):
                        for dt in range(DT):
                            tag, tb = scr_tags[dt]
                            scr = tp.tile([P, L], bf16, tag=tag, bufs=tb,
                                          name=f"tscr_{b}_{dt}")
                            eng = nc.gpsimd if dt % 2 == 0 else nc.vector
                            eng.tensor_mul(scr, hTn_s[:, dt, :], st["rbc"])
                            scrs.append(scr)
                        for dt in range(DT):
                            nc.vector.tensor_reduce(mv_s[:, dt, b:b + 1],
                                                    scrs[dt], axis=AXX,
                                                    op=ALU.add)
                            if fink:
                                fink(dt, dt == DT - 1)

                return tail_a, tail_b, tail_cx, tail_ch, tail_final

            pending = None
            nxt = None
            fin = {}
            nonlocal_state = {}
            for b in range(B):
                if nxt is None:
                    xT_s = xtp.tile([P, DT, L], f32r, tag="xT")
                    # per-chunk interleave: ph1's dk=0 matmuls start after
                    # only w1t[0]+xT[0] (~0.75MB) instead of half of both
                    for dk in range(DT):
                        nc.sync.dma_start(out=w1t_s[:, dk:dk + 1, :],
                                          in_=w1t_ext[dk * P:(dk + 1) * P, :]
                                          .rearrange("(k p) e -> p k e", p=P))
                        nc.sync.dma_start(out=xT_s[:, dk:dk + 1, :],
                                          in_=xT_ext[b, dk * P:(dk + 1) * P, :]
                                          .rearrange("(k p) l -> p k l", p=P))
                    x_s = xp.tile([P, LT, D], bf16, tag="x")
                    nc.sync.dma_start(
                        out=x_s, in_=x_ext[b].rearrange("(k p) d -> p k d", p=P))
                    # bufs=1: x8(b)'s last read (ph3dr lc1) lands well before
                    # the next item's prefetch DMA
                    x8_s = xp.tile([P, LT, D], fp8, tag="x8", bufs=1)
                    nc.sync.dma_start(
                        out=x8_s, in_=x8_ext[b].rearrange("(k p) d -> p k d", p=P))
                    nc.sync.dma_start(
                        out=wo1t8_s, in_=wo1t8_ext.rearrange("(k p) e -> p k e", p=P))
                    nc.sync.dma_start(
                        out=w2t_s, in_=w2t_ext.rearrange("(k p) e -> p k e", p=P))
                    nc.sync.dma_start(
                        out=wo2t_s, in_=wo2t_ext.rearrange("(k p) e -> p k e", p=P))
                    qT0, q8T0 = emit_ph1(xT_s, b, 0)
                else:
                    xT_s, x_s, x8_s, qT0, q8T0 = nxt
                    nxt = None
                hTn_s = hp.tile([P, DT, L], f32r, tag="hTn")
                q2red_s = vp.tile([P, D2T, NLC], f32, tag="q2red", bufs=1,
                                  name=f"q2red_{b}")

                # ---- stage 1.  Emission order keeps every DVE chain under
                # PE cover: ph1(lc1)+ph2(lc1) matmuls cover the A8(lc0)
                # normalize chain; ph3/ph4-DR of lc0 + lc1 cover the ph5(lc0)
                # and A8(lc1) chains; ph7(lc0) covers ph5(lc1)+tail_c.
                exp0, psd0 = emit_ph2(xT_s, qT0, b, 0)
                qT1, q8T1 = emit_ph1(xT_s, b, 1)
                bcb0 = bc_recip(psd0, b, 0)
                A8_0 = emit_normA8(exp0, bcb0, b, 0)
                exp1, psd1 = emit_ph2(xT_s, qT1, b, 1)
                if pending:
                    pending[0]()                    # r row extraction (PE+ACT)
                    pending[1]()                    # r broadcast (PE+ACT)
                # A8(lc1) chain ahead of ph5(lc0)'s in the DVE FIFO: it only
                # waits on psd1 and must not queue behind the norm chain
                # (ph3dr(lc1) would stall on it)
                bcb1 = bc_recip(psd1, b, 1)
                A8_1 = emit_normA8(exp1, bcb1, b, 1)
                mix8_0 = emit_ph3_dr(x8_s, A8_0, b, 0)
                emit_ph4_dr(mix8_0, q8T0, hTn_s, 0)
                emit_ph5(hTn_s, b, 0)
                mix8_1 = emit_ph3_dr(x8_s, A8_1, b, 1)
                emit_ph4_dr(mix8_1, q8T1, hTn_s, 1)
                emit_ph5(hTn_s, b, 1)
                if pending:
                    pending[2]()                    # x-part row-sums (PE+DVE)

                if b < B - 1:
                    def head_dma(bn=b + 1):
                        xTn = xtp.tile([P, DT, L], f32r, tag="xT")
                        nc.sync.dma_start(
                            out=xTn[:, 0:2, :],
                            in_=xT_ext[bn, 0:2 * P, :]
                            .rearrange("(k p) l -> p k l", p=P))
                        nc.sync.dma_start(
                            out=xTn[:, 2:DT, :],
                            in_=xT_ext[bn, 2 * P:DT * P, :]
                            .rearrange("(k p) l -> p k l", p=P))
                        xn = xp.tile([P, LT, D], bf16, tag="x")
                        nc.sync.dma_start(
                            out=xn,
                            in_=x_ext[bn].rearrange("(k p) d -> p k d", p=P))
                        x8n = xp.tile([P, LT, D], fp8, tag="x8", bufs=1)
                        nc.sync.dma_start(
                            out=x8n,
                            in_=x8_ext[bn].rearrange("(k p) d -> p k d", p=P))
                        nonlocal_state["dma"] = (xTn, xn, x8n)

                    def head_ph1(bn=b + 1):
                        xTn, xn, x8n = nonlocal_state.pop("dma")
                        qn, q8n = emit_ph1(xTn, bn, 0)
                        nonlocal_state["nxt"] = (xTn, xn, x8n, qn, q8n)
                    last_fill = head_dma
                else:
                    def last_fill():
                        with nc.allow_low_precision(reason="f32r sums"):
                            nc.vector.tensor_reduce(mv_s[:, D2T:C2T, b:b + 1],
                                                    q2red_s, axis=AXX,
                                                    op=ALU.add)
                        emb_ps = pp.tile([P, LC], f32, tag="ps", name="emb_ps")
                        for i, ck in enumerate(range(D2T, C2T)):
                            mm(emb_ps[0:B, :], mv_s[:, ck, :], wo2t_s[:, ck, :],
                               i == 0, False)
                        fin["emb_ps"] = emb_ps

                # ---- stage 2 (ph7 of lc1 sliced into ph8(lc0)'s chain
                # shadows; the prev item's weighted row-sums (tail_ch) spread
                # one per ph7/ph8 block; the next item's DMA+ph1 (or the
                # final q2-column matmuls) slice into ph8(lc1)'s last shadow)
                rrow_ps = [pp.tile([P, LC], f32, tag="ps", name=f"rrow_{b}_{i}")
                           for i in range(NLC)]
                q2T0 = tp.tile([P, D2T, LC], f32r, tag="q2", name=f"q2_{b}_0")
                emit_ph7(hTn_s, xT_s, q2red_s, q2T0, 0, 0, 4)
                if pending:
                    pending[3](0)
                emit_ph7(hTn_s, xT_s, q2red_s, q2T0, 0, 4, D2T)
                if pending:
                    pending[3](1)
                pu0 = emit_ph8_nat(hTn_s, xT_s, q2T0, rrow_ps, b, 0)
                q2T1 = tp.tile([P, D2T, LC], f32r, tag="q2", name=f"q2_{b}_1")
                emit_ph7(hTn_s, xT_s, q2red_s, q2T1, 1, 0, 2)
                pu0()
                if b < B - 1:
                    # next item's DMA + ph1 up here: its qT copies must land
                    # in the DVE/ACT FIFOs before the tail pieces' reduces,
                    # or next item's ph2(lc0) stalls on them; the DMA also
                    # gains ~12us of runway
                    last_fill()
                if pending:
                    pending[3](2)
                emit_ph7(hTn_s, xT_s, q2red_s, q2T1, 1, 2, D2T)
                if pending:
                    pending[3](3)
                    pending = None
                if b < B - 1:
                    head_ph1()

                pu1 = emit_ph8_nat(hTn_s, xT_s, q2T1, rrow_ps, b, 1)
                if b == B - 1:
                    last_fill()     # needs the full q2red: must emit late
                pu1()
                if b < B - 1:
                    nxt = nonlocal_state.pop("nxt")

                def fink(ck, last, bb=b):
                    if bb == B - 1:
                        mm(fin["emb_ps"][0:B, :], mv_s[:, ck, :],
                           wo2t_s[:, ck, :], False, last)

                pending = make_tail(b, hTn_s, xT_s, x_s, rrow_ps, q2red_s,
                                    do_q2mv=(b < B - 1),
                                    fink=fink if b == B - 1 else None)

            # last item's tail (final Wo2 matmuls ride inside via fink)
            pending[0]()
            pending[1]()
            pending[2]()
            pending[4]()
            emb_ps = fin["emb_ps"]
            embf_s = vp.tile([B, D], f32, tag="bc", bufs=2, name="embf")
            nc.scalar.copy(embf_s, emb_ps[0:B, :])
            nc.sync.dma_start(out=out_ext[:, :], in_=embf_s)

    _t1 = _time.time()
    nc.compile()
    print(f"[kernel] tile-trace+schedule {_t1 - _t0:.1f}s, "
          f"bacc compile {_time.time() - _t1:.1f}s", file=sys.stderr, flush=True)
    return nc


def get_nc():
    # the pipelined item tail reads xT(b) during item b+1, so the xT pool
    # MUST be double-buffered -- no xt_bufs=1 fallback (it deadlocks)
    if "nc" not in _CACHE:
        _CACHE["nc"] = _build_nc(xt_bufs=2)
    return _CACHE["nc"]


def make_in_maps(x, W1, Wo1, W2, Wo2):
    import ml_dtypes
    x = np.ascontiguousarray(np.asarray(x, dtype=np.float32))
    xT = np.ascontiguousarray(x.transpose(0, 2, 1))
    x_bf = np.ascontiguousarray(x.astype(ml_dtypes.bfloat16))
    x_f8 = np.ascontiguousarray(
        np.clip(x, -240, 240).astype(ml_dtypes.float8_e4m3))
    w1t = np.ascontiguousarray(np.asarray(W1, np.float32).T)
    # Wo1 in fp8 with a x64 power-of-2 scale (values ~0.02 land in e4m3's
    # sweet spot); the ph4 tanh applies scale=1/64
    wo1t8 = np.ascontiguousarray(
        np.clip(np.asarray(Wo1, np.float32).T * 64.0, -240, 240)
        .astype(ml_dtypes.float8_e4m3))
    w2t = np.ascontiguousarray(np.asarray(W2, np.float32).T)
    # 1/L mean-scale folded into Wo2 (it only feeds the final matmuls)
    wo2t = np.ascontiguousarray((np.asarray(Wo2, np.float32).T / L).astype(ml_dtypes.bfloat16))
    onesm = np.ones((P, P), dtype=ml_dtypes.bfloat16)
    onesr = np.ones((1, P), dtype=ml_dtypes.bfloat16)
    return [
        {"x": x_bf[c * B:(c + 1) * B], "x8": x_f8[c * B:(c + 1) * B],
         "xT": xT[c * B:(c + 1) * B],
         "w1t": w1t, "wo1t8": wo1t8, "w2t": w2t, "wo2t": wo2t,
         "onesm": onesm, "onesr": onesr}
        for c in range(NCORES)
    ]


def run(x, W1, Wo1, W2, Wo2, trace=False, **kw):
    from concourse.bass_utils import run_bass_kernel_spmd
    nc = get_nc()
    in_maps = make_in_maps(x, W1, Wo1, W2, Wo2)
    res = run_bass_kernel_spmd(nc, in_maps, core_ids=list(range(NCORES)),
                               trace=trace, **kw)
    out = np.concatenate([res.results[c]["out"] for c in range(NCORES)], axis=0)
    return out.reshape(N_GLOBAL, D, 1, 1), res


def kernel(**inputs):
    out, _ = run(inputs["x"], inputs["W1"], inputs["Wo1"],
                 inputs["W2"], inputs["Wo2"])
    return out


# revision 42
# speedup vs baseline: 1.0014x; 1.0014x over previous
"""AttentionFuserV3 Trainium2 kernel: 8-core pure data parallel over batch.

v2: fp8 DoubleRow on the probability/output matmuls of stage 1.

Reference computation per batch item x_b [L=1024, D=512]:
  stage1: q = x W1^T; S = q x^T; A = softmax(S); mix = A x;
          h = tanh([mix, q] Wo1^T); h = h / max(||h||_2, eps)     (per row)
  stage2: c = [h, x]; q2 = c W2^T; S2 = q2 c^T; A2 = softmax(S2);
          mix2 = A2 c; o = [mix2, q2] Wo2^T; emb = mean_l(o)

Layout: T-space for stage 1 (features on partitions), natural orientation
for stage-2 attention (see ph8).  Stage-2 exploits linearity of the final
mean (r-trick): emb = (1/L)[c^T r ; sum_l q2] Wo2^T.

fp8 (e4m3) with perf_mode=DoubleRow (K=256 per instruction, ~2x PE
throughput at N=512) is used where measured end-to-end error stays small:
  - ph3 (mix = A x): probs are normalized to [0,1] on DVE (exp_bf16 x
    recip -> fp8) and x ships as fp8 from the host.  (3.1e-3 alone)
  - ph4 (out1 = [mix,q] Wo1^T): Wo1 ships as fp8 x64 (tanh applies
    scale=1/64), mix/q are fp8 copies of their psums.  (4.3e-3 alone)
Scores, W1/W2 projections and stage 2 stay f32r: fp8 there measured
1e-2..8e-2 end-to-end (softmax logit noise) -- over the 2e-2 gate.

DVE relief: the stage-2 softmax denominators (dsum) and the q2 column
sums (q2red) ride the ACT accumulator (accum_out) of the exp /
psum-copy instructions instead of DVE tensor_reduces.
"""

import sys

sys.path.insert(0, "/opt/trn_rl_repo")

import numpy as np

N_GLOBAL, L, D = 32, 1024, 512
NCORES = 8
B = N_GLOBAL // NCORES          # 4 batch items per core
P = 128
LC = 512                        # l-chunk (matmul moving free dim)
NLC = L // LC                   # 2
DT = D // P                     # 4
LT = L // P                     # 8
D2T = 2 * D // P                # 8
C2T = 4 * D // P                # 16

_CACHE = {}


def _build_nc(xt_bufs=2):
    import concourse.bass as bass  # noqa: F401
    import concourse.mybir as mybir
    import concourse.tile as tile
    from concourse import bacc

    f32 = mybir.dt.float32
    f32r = mybir.dt.float32r
    bf16 = mybir.dt.bfloat16
    fp8 = mybir.dt.float8e4
    AF = mybir.ActivationFunctionType
    ALU = mybir.AluOpType
    AXX = mybir.AxisListType.X
    DR = mybir.MatmulPerfMode.DoubleRow

    nc = bacc.Bacc("TRN2", target_bir_lowering=False, debug=False,
                   num_devices=NCORES)

    x_ext = nc.declare_dram_parameter("x", [B, L, D], bf16, isOutput=False)
    x8_ext = nc.declare_dram_parameter("x8", [B, L, D], fp8, isOutput=False)
    xT_ext = nc.declare_dram_parameter("xT", [B, D, L], f32r, isOutput=False)
    w1t_ext = nc.declare_dram_parameter("w1t", [D, D], f32r, isOutput=False)
    wo1t8_ext = nc.declare_dram_parameter("wo1t8", [2 * D, D], fp8, isOutput=False)
    w2t_ext = nc.declare_dram_parameter("w2t", [2 * D, 2 * D], f32r, isOutput=False)
    wo2t_ext = nc.declare_dram_parameter("wo2t", [4 * D, D], bf16, isOutput=False)
    # Constants shipped from host: walrus's ISA check rejects memset/iota
    # writes into float32r tiles, but DMA from an f32r DRAM param is fine.
    onm_ext = nc.declare_dram_parameter("onesm", [P, P], bf16, isOutput=False)
    onr_ext = nc.declare_dram_parameter("onesr", [1, P], bf16, isOutput=False)
    out_ext = nc.declare_dram_parameter("out", [B, D], f32, isOutput=True)

    import time as _time
    _t0 = _time.time()
    with tile.TileContext(nc) as tc:
        with tc.tile_pool(name="wp", bufs=1) as wp, \
             tc.tile_pool(name="cp", bufs=1) as cp, \
             tc.tile_pool(name="xtp", bufs=xt_bufs) as xtp, \
             tc.tile_pool(name="xp", bufs=2) as xp, \
             tc.tile_pool(name="hp", bufs=2) as hp, \
             tc.tile_pool(name="tp", bufs=1) as tp, \
             tc.tile_pool(name="vp", bufs=2) as vp, \
             tc.tile_pool(name="ep", bufs=1) as ep, \
             tc.tile_pool(name="ps", bufs=8, space="PSUM") as pp:

            # ---- w1t gates ph1 of item 0: DMA it first (chunks of it
            # interleave with the xT chunks inside the b==0 iteration)
            w1t_s = wp.tile([P, DT, D], f32r, tag="w1t")
            onesm_s = cp.tile([P, P], bf16, tag="onesm")
            nc.sync.dma_start(out=onesm_s, in_=onm_ext[:, :])
            onesr_s = cp.tile([1, P], bf16, tag="onesr")
            nc.sync.dma_start(out=onesr_s, in_=onr_ext[:, :])

            wo1t8_s = wp.tile([P, D2T, D], fp8, tag="wo1t8")
            w2t_s = wp.tile([P, D2T, 2 * D], f32r, tag="w2t")
            wo2t_s = wp.tile([P, C2T, D], bf16, tag="wo2t")

            # meanvec columns for the deferred Wo2 projection:
            # c-chunks 0..3 = sum_l mix2 (h part), 4..7 (x part), 8..15 = sum_l q2
            mv_s = ep.tile([P, C2T, B], bf16, tag="mv")

            def mm(out, lhsT, rhs, first, last, pm=None):
                nc.tensor.matmul(out, lhsT, rhs, start=first, stop=last,
                                 perf_mode=pm)

            def bc_recip(denom_ps, b, lc):
                """[128,512] PSUM pre-broadcast softmax denominator ->
                f32 + bf16 approx reciprocals."""
                bc = vp.tile([P, LC], f32, tag="bc", bufs=2,
                             name=f"bc_{b}_{lc}")
                bcb = vp.tile([P, LC], bf16, tag="bcb", bufs=1,
                              name=f"bcb_{b}_{lc}")
                nc.vector.reciprocal_approx_fast(out=bc, in_=denom_ps)
                with nc.allow_low_precision(reason="bf16 softmax scale"):
                    nc.vector.tensor_copy(bcb, bc)
                return bcb

            def emit_normA8(expT_s, bcb, b, lc):
                """Normalized attention probs in fp8: A8 = expT * (1/denom)."""
                A8_s = tp.tile([P, LT, LC], fp8, tag="a8", bufs=2,
                               name=f"a8_{b}_{lc}")
                with nc.allow_low_precision(reason="fp8 probs"):
                    for mt in range(LT):
                        nc.vector.tensor_mul(A8_s[:, mt, :], expT_s[:, mt, :],
                                             bcb)
                return A8_s

            def emit_ph1(xT_s, b, lc):
                """q chunks: f32r copy (ph2 rhs) + fp8 copy (ph4 rhs)."""
                ls = slice(lc * LC, (lc + 1) * LC)
                qT_s = tp.tile([P, DT, LC], f32r, tag="qt", bufs=1,
                               name=f"qt_{b}_{lc}")
                q8T_s = tp.tile([P, DT, LC], fp8, tag="q8", bufs=2,
                                name=f"q8_{b}_{lc}")
                pss = []
                for et in range(DT):
                    ps = pp.tile([P, LC], f32, tag="ps")
                    for dk in range(DT):
                        mm(ps, w1t_s[:, dk, et * P:(et + 1) * P],
                           xT_s[:, dk, ls], dk == 0, dk == DT - 1)
                    # qT alternates ACT/DVE (ph2 needs it soon); q8 copies
                    # all ride DVE afterwards -- ACT must stay clear for the
                    # next ph2's exps (the deferred denominator matmuls
                    # stall the PE on them)
                    with nc.allow_low_precision(reason="f32r stores"):
                        if et % 2 == 0:
                            nc.scalar.copy(qT_s[:, et, :], ps)
                        else:
                            nc.vector.tensor_copy(qT_s[:, et, :], ps)
                    pss.append(ps)
                with nc.allow_low_precision(reason="fp8 stores"):
                    for et in range(DT):
                        nc.vector.tensor_copy(q8T_s[:, et, :], pss[et])
                return qT_s, q8T_s

            def emit_ph2(xT_s, qT_s, b, lc):
                expT_s = tp.tile([P, LT, LC], bf16, tag="exp",
                                 name=f"exp1_{b}_{lc}")
                ps_d = pp.tile([P, LC], f32, tag="ps")
                # denominator accumulation deferred TWO mt so each
                # ones-matmul hides two tiles behind its exp: the ACT queue
                # (copies ahead of the exps) gets ~2us of slack before the
                # PE stream would stall on a denominator's exp
                for mt in range(LT):
                    ps = pp.tile([P, LC], f32, tag="ps")
                    for ek in range(DT):
                        mm(ps, xT_s[:, ek, mt * P:(mt + 1) * P],
                           qT_s[:, ek, :], ek == 0, ek == DT - 1)
                    if mt > 1:
                        mm(ps_d, onesm_s, expT_s[:, mt - 2, :],
                           mt - 2 == 0, False)
                    nc.scalar.activation(expT_s[:, mt, :], ps, AF.Exp)
                mm(ps_d, onesm_s, expT_s[:, LT - 2, :], False, False)
                mm(ps_d, onesm_s, expT_s[:, LT - 1, :], False, True)
                return expT_s, ps_d

            def emit_ph3_dr(x8_s, A8_s, b, lc):
                """mix = A x as fp8 DoubleRow over m-chunk pairs; A8 is
                normalized so the psum is the final mix -> fp8 copy."""
                mix8_s = tp.tile([P, DT, LC], fp8, tag="mix8", bufs=1,
                                 name=f"mix8_{b}_{lc}")
                for dt in range(DT):
                    ps = pp.tile([P, LC], f32, tag="ps")
                    for mk in range(0, LT, 2):
                        mm(ps, x8_s[:, mk:mk + 2, dt * P:(dt + 1) * P],
                           A8_s[:, mk:mk + 2, :], mk == 0, mk == LT - 2,
                           pm=DR)
                    with nc.allow_low_precision(reason="fp8 mix store"):
                        nc.scalar.copy(mix8_s[:, dt, :], ps)
                return mix8_s

            def emit_ph4_dr(mix8_s, q8T_s, hTn_s, lc):
                """out1 -> tanh(psum/64) via fp8 DoubleRow over c-chunk
                pairs (Wo1 ships as fp8 x64)."""
                ls = slice(lc * LC, (lc + 1) * LC)
                for ot in range(DT):
                    ps = pp.tile([P, LC], f32, tag="ps")
                    for ck in range(0, D2T, 2):
                        rhs = (mix8_s[:, ck:ck + 2, :] if ck < DT
                               else q8T_s[:, ck - DT:ck - DT + 2, :])
                        mm(ps, wo1t8_s[:, ck:ck + 2, ot * P:(ot + 1) * P],
                           rhs, ck == 0, ck == D2T - 2, pm=DR)
                    nc.scalar.activation(hTn_s[:, ot, ls], ps, AF.Tanh,
                                         scale=1.0 / 64)

            def emit_ph5(hTn_s, b, lc):
                """L2-normalize hTn[:, :, ls] in place (norm over the
                partition axis via ones-matmul)."""
                ls = slice(lc * LC, (lc + 1) * LC)
                # hsq = 16*h^2 in fp8 (the x16 keeps h^2 out of e4m3's
                # subnormal range, where flushing biases the norm low);
                # the sqrt's scale=1/16 undoes it exactly
                hsq_s = tp.tile([P, DT, LC], fp8, tag="hsq", name=f"hsq_{b}_{lc}")
                with nc.allow_low_precision(reason="fp8 16*h^2 for norm"):
                    for dt in range(DT):
                        nc.vector.scalar_tensor_tensor(
                            hsq_s[:, dt, :], hTn_s[:, dt, ls], 16.0,
                            hTn_s[:, dt, ls], op0=ALU.mult, op1=ALU.mult)
                ps_n = pp.tile([P, LC], f32, tag="ps")
                for dt in range(DT):
                    mm(ps_n, onesm_s, hsq_s[:, dt, :], dt == 0, dt == DT - 1)
                bcn = vp.tile([P, LC], f32, tag="bc", bufs=2, name=f"bcn_{b}_{lc}")
                bc2 = vp.tile([P, LC], f32, tag="bc", bufs=2, name=f"bc2_{b}_{lc}")
                nc.scalar.activation(bcn, ps_n, AF.Sqrt, scale=1.0 / 16)
                nc.vector.tensor_scalar_max(bcn, bcn, 1e-12)
                nc.vector.reciprocal_approx_fast(out=bc2, in_=bcn)
                for dt in range(DT):
                    nc.vector.tensor_mul(hTn_s[:, dt, ls], hTn_s[:, dt, ls], bc2)

            def emit_ph7(hTn_s, xT_s, q2red_s, q2T_s, lc, et_lo, et_hi):
                ls = slice(lc * LC, (lc + 1) * LC)

                def c2T(k, fs):
                    return hTn_s[:, k, fs] if k < DT else xT_s[:, k - DT, fs]

                for et in range(et_lo, et_hi):
                    ps = pp.tile([P, LC], f32, tag="ps")
                    for dk in range(D2T):
                        mm(ps, w2t_s[:, dk, et * P:(et + 1) * P],
                           c2T(dk, ls), dk == 0, dk == D2T - 1)
                    # psum->SBUF copy accumulates the q2 column-sum partial
                    # on the ACT accumulator: a DVE tensor_reduce here would
                    # read f32r (2x slow path, ~1.1us) and clog the DVE FIFO
                    # ahead of ph8's u-chain
                    nc.scalar.activation(q2T_s[:, et, :], ps, AF.Copy,
                                         accum_out=q2red_s[:, et, lc:lc + 1])

            def emit_ph8_nat(hTn_s, xT_s, q2T_s, rrow_ps, b, lc):
                """Stage-2 attention in NATURAL orientation (query l on
                partitions): per l-tile, the softmax denominator rides the
                exp's ACT accumulator and r accumulates via matmuls with the
                reciprocal vector as lhsT -- r = sum_lt u_lt^T @ exp2n_lt."""
                def c2T(k, fs):
                    return hTn_s[:, k, fs] if k < DT else xT_s[:, k - DT, fs]

                pend_u = [None]

                def flush_u():
                    if pend_u[0] is not None:
                        pu_b, pe2n, plt = pend_u[0]
                        pend_u[0] = None
                        for ms in range(NLC):
                            mm(rrow_ps[ms][0:1, :], pu_b, pe2n[:, ms, :],
                               plt == 0, plt == LT - 1)

                for li in range(LT // NLC):
                    lt = lc * (LT // NLC) + li
                    loff = li * P
                    e2n_s = tp.tile([P, NLC, LC], bf16, tag="e2n",
                                    name=f"e2n_{b}_{lt}", bufs=2)
                    dsum = vp.tile([P, 3], f32, tag="dsum", bufs=3,
                                   name=f"dsum_{b}_{lt}")
                    ps2 = [pp.tile([P, LC], f32, tag="ps",
                                   name=f"ps8_{b}_{lt}_{i}") for i in range(NLC)]
                    for ek in range(D2T):
                        for ms in range(NLC):
                            mm(ps2[ms], q2T_s[:, ek, loff:loff + P],
                               c2T(ek, slice(ms * LC, (ms + 1) * LC)),
                               ek == 0, ek == D2T - 1)
                    flush_u()   # previous lt's u-matmuls, now chain-covered
                    for ms in range(NLC):
                        nc.scalar.activation(e2n_s[:, ms, :], ps2[ms], AF.Exp,
                                             accum_out=dsum[:, ms:ms + 1])
                    nc.vector.tensor_reduce(dsum[:, 2:3], dsum[:, 0:2],
                                            axis=AXX, op=ALU.add)
                    u_f = vp.tile([P, 1], f32, tag="uf", bufs=3,
                                  name=f"uf_{b}_{lt}")
                    u_b = vp.tile([P, 1], bf16, tag="ub", bufs=3,
                                  name=f"ub_{b}_{lt}")
                    nc.vector.reciprocal_approx_fast(out=u_f, in_=dsum[:, 2:3])
                    with nc.allow_low_precision(reason="bf16 softmax scale"):
                        nc.vector.tensor_copy(u_b, u_f)
                    pend_u[0] = (u_b, e2n_s, lt)
                return flush_u

            def make_tail(b, hTn_s, xT_s, x_s, rrow_ps, q2red_s,
                          do_q2mv=True, fink=None):
                """Item tail, split in three so it can be emitted interleaved
                into the next item's stage-1 engine streams."""
                st = {}

                def tail_a():
                    if do_q2mv:
                        with nc.allow_low_precision(reason="f32r sums"):
                            nc.vector.tensor_reduce(mv_s[:, D2T:C2T, b:b + 1],
                                                    q2red_s, axis=AXX,
                                                    op=ALU.add)
                    rflat_s = vp.tile([1, L], bf16, tag="rflat", bufs=1,
                                      name=f"rflat_{b}")
                    nc.scalar.copy(rflat_s[0:1, 0:LC], rrow_ps[0][0:1, :])
                    nc.scalar.copy(rflat_s[0:1, LC:L], rrow_ps[1][0:1, :])
                    # r row -> column chunks: K=1 matmuls into disjoint
                    # columns of one psum bank
                    rc_ps = pp.tile([P, LT], f32, tag="ps", name=f"rc_{b}")
                    for mt in range(LT):
                        mm(rc_ps[:, mt:mt + 1],
                           rflat_s[0:1, mt * P:(mt + 1) * P],
                           onesr_s[0:1, 0:1], mt == 0, mt == LT - 1)
                    rsum_s = vp.tile([P, LT], bf16, tag="rsum", bufs=1,
                                     name=f"rsum_{b}")
                    with nc.allow_low_precision(reason="bf16 r"):
                        nc.vector.tensor_copy(rsum_s, rc_ps)
                    st["rflat"] = rflat_s
                    st["rsum"] = rsum_s

                def tail_b():
                    rbc_s = vp.tile([P, L], bf16, tag="rbc", bufs=1,
                                    name=f"rbc_{b}")
                    for j in range(NLC):
                        ps_b = pp.tile([P, LC], f32, tag="ps")
                        mm(ps_b, onesr_s, st["rflat"][0:1, j * LC:(j + 1) * LC],
                           True, True)
                        nc.scalar.copy(rbc_s[:, j * LC:(j + 1) * LC], ps_b)
                    st["rbc"] = rbc_s

                def tail_cx():
                    rsum_s = st["rsum"]
                    with nc.allow_low_precision(reason="f32r rounding of sums"):
                        # x part: sum_m x[m,d] r[m] as tiny bf16 matmuls
                        for dt in range(DT):
                            ps_x = pp.tile([P, 1], f32, tag="ps",
                                           name=f"psx_{b}_{dt}")
                            for mk in range(LT):
                                mm(ps_x, x_s[:, mk, dt * P:(dt + 1) * P],
                                   rsum_s[:, mk:mk + 1], mk == 0, mk == LT - 1)
                            nc.vector.tensor_copy(mv_s[:, DT + dt, b:b + 1],
                                                  ps_x)
                            if fink:
                                fink(DT + dt, False)

                def tail_ch(dt):
                    # h part: one weighted row-sum per call -- multiply on
                    # GpSimd (pure slack) into a bf16 scratch, reduce the
                    # bf16 on DVE (f32r reduces hit a 2x slow path and the
                    # in-place f32r write chained Pool behind DVE).
                    # Scratch rides the hsq tag: hsq lives in stage 1, tscr
                    # in stage 2 -- strictly sequential lifetimes.
                    scr = tp.tile([P, L], bf16, tag="hsq", bufs=1,
                                  name=f"tscr_{b}_{dt}")
                    with nc.allow_low_precision(reason="bf16 tail products"):
                        nc.gpsimd.tensor_mul(scr, hTn_s[:, dt, :], st["rbc"])
                        nc.vector.tensor_reduce(mv_s[:, dt, b:b + 1],
                                                scr, axis=AXX, op=ALU.add)
                    if fink:
                        fink(dt, dt == DT - 1)

                def tail_final():
                    # end-of-kernel drain: nothing covers this, so run the
                    # four multiplies two-per-engine first, then the reduces
                    # (+ the final Wo2 matmuls via fink) in order
                    scrs = []
                    # one scratch per dead stage-1 tag so all four products
                    # can be outstanding at once
                    scr_tags = [("hsq", 1), ("exp", 1), ("a8", 2), ("mix8", 1)]
                    with nc.allow_low_precision(reason="bf16 tail products"):
                        for dt in range(DT):
                            tag, tb = scr_tags[dt]
                            scr = tp.tile([P, L], bf16, tag=tag, bufs=tb,
                                          name=f"tscr_{b}_{dt}")
                            eng = nc.gpsimd if dt % 2 == 0 else nc.vector
                            eng.tensor_mul(scr, hTn_s[:, dt, :], st["rbc"])
                            scrs.append(scr)
                        for dt in range(DT):
                            nc.vector.tensor_reduce(mv_s[:, dt, b:b + 1],
                                                    scrs[dt], axis=AXX,
                                                    op=ALU.add)
                            if fink:
                                fink(dt, dt == DT - 1)

                return tail_a, tail_b, tail_cx, tail_ch, tail_final

            pending = None
            nxt = None
            fin = {}
            nonlocal_state = {}
            for b in range(B):
                if nxt is None:
                    xT_s = xtp.tile([P, DT, L], f32r, tag="xT")
                    # per-chunk interleave: ph1's dk=0 matmuls start after
                    # only w1t[0]+xT[0] (~0.75MB) instead of half of both
                    for dk in range(DT):
                        nc.sync.dma_start(out=w1t_s[:, dk:dk + 1, :],
                                          in_=w1t_ext[dk * P:(dk + 1) * P, :]
                                          .rearrange("(k p) e -> p k e", p=P))
                        nc.sync.dma_start(out=xT_s[:, dk:dk + 1, :],
                                          in_=xT_ext[b, dk * P:(dk + 1) * P, :]
                                          .rearrange("(k p) l -> p k l", p=P))
                    x_s = xp.tile([P, LT, D], bf16, tag="x")
                    nc.sync.dma_start(
                        out=x_s, in_=x_ext[b].rearrange("(k p) d -> p k d", p=P))
                    # bufs=1: x8(b)'s last read (ph3dr lc1) lands well before
                    # the next item's prefetch DMA
                    x8_s = xp.tile([P, LT, D], fp8, tag="x8", bufs=1)
                    nc.sync.dma_start(
                        out=x8_s, in_=x8_ext[b].rearrange("(k p) d -> p k d", p=P))
                    nc.sync.dma_start(
                        out=wo1t8_s, in_=wo1t8_ext.rearrange("(k p) e -> p k e", p=P))
                    nc.sync.dma_start(
                        out=w2t_s, in_=w2t_ext.rearrange("(k p) e -> p k e", p=P))
                    nc.sync.dma_start(
                        out=wo2t_s, in_=wo2t_ext.rearrange("(k p) e -> p k e", p=P))
                    qT0, q8T0 = emit_ph1(xT_s, b, 0)
                else:
                    xT_s, x_s, x8_s, qT0, q8T0 = nxt
                    nxt = None
                hTn_s = hp.tile([P, DT, L], f32r, tag="hTn")
                q2red_s = vp.tile([P, D2T, NLC], f32, tag="q2red", bufs=1,
                                  name=f"q2red_{b}")

                # ---- stage 1.  Emission order keeps every DVE chain under
                # PE cover: ph1(lc1)+ph2(lc1) matmuls cover the A8(lc0)
                # normalize chain; ph3/ph4-DR of lc0 + lc1 cover the ph5(lc0)
                # and A8(lc1) chains; ph7(lc0) covers ph5(lc1)+tail_c.
                ns = nc.named_scope
                with ns("ph2.0"):
                    exp0, psd0 = emit_ph2(xT_s, qT0, b, 0)
                with ns("ph1.1"):
                    qT1, q8T1 = emit_ph1(xT_s, b, 1)
                with ns("normA8.0"):
                    bcb0 = bc_recip(psd0, b, 0)
                    A8_0 = emit_normA8(exp0, bcb0, b, 0)
                with ns("ph2.1"):
                    exp1, psd1 = emit_ph2(xT_s, qT1, b, 1)
                if pending:
                    with ns("tail_a"):
                        pending[0]()                # r row extraction (PE+ACT)
                    with ns("tail_b"):
                        pending[1]()                # r broadcast (PE+ACT)
                # A8(lc1) chain ahead of ph5(lc0)'s in the DVE FIFO: it only
                # waits on psd1 and must not queue behind the norm chain
                # (ph3dr(lc1) would stall on it)
                with ns("normA8.1"):
                    bcb1 = bc_recip(psd1, b, 1)
                    A8_1 = emit_normA8(exp1, bcb1, b, 1)
                with ns("ph3dr.0"):
                    mix8_0 = emit_ph3_dr(x8_s, A8_0, b, 0)
                with ns("ph4dr.0"):
                    emit_ph4_dr(mix8_0, q8T0, hTn_s, 0)
                with ns("ph5.0"):
                    emit_ph5(hTn_s, b, 0)
                with ns("ph3dr.1"):
                    mix8_1 = emit_ph3_dr(x8_s, A8_1, b, 1)
                with ns("ph4dr.1"):
                    emit_ph4_dr(mix8_1, q8T1, hTn_s, 1)
                with ns("ph5.1"):
                    emit_ph5(hTn_s, b, 1)
                if pending:
                    with ns("tail_cx"):
                        pending[2]()                # x-part row-sums (PE+DVE)

                if b < B - 1:
                    def head_dma(bn=b + 1):
                        xTn = xtp.tile([P, DT, L], f32r, tag="xT")
                        nc.sync.dma_start(
                            out=xTn[:, 0:2, :],
                            in_=xT_ext[bn, 0:2 * P, :]
                            .rearrange("(k p) l -> p k l", p=P))
                        nc.sync.dma_start(
                            out=xTn[:, 2:DT, :],
                            in_=xT_ext[bn, 2 * P:DT * P, :]
                            .rearrange("(k p) l -> p k l", p=P))
                        xn = xp.tile([P, LT, D], bf16, tag="x")
                        nc.sync.dma_start(
                            out=xn,
                            in_=x_ext[bn].rearrange("(k p) d -> p k d", p=P))
                        x8n = xp.tile([P, LT, D], fp8, tag="x8", bufs=1)
                        nc.sync.dma_start(
                            out=x8n,
                            in_=x8_ext[bn].rearrange("(k p) d -> p k d", p=P))
                        nonlocal_state["dma"] = (xTn, xn, x8n)

                    def head_ph1(bn=b + 1):
                        xTn, xn, x8n = nonlocal_state.pop("dma")
                        qn, q8n = emit_ph1(xTn, bn, 0)
                        nonlocal_state["nxt"] = (xTn, xn, x8n, qn, q8n)
                    last_fill = head_dma
                else:
                    def last_fill():
                        with nc.allow_low_precision(reason="f32r sums"):
                            nc.vector.tensor_reduce(mv_s[:, D2T:C2T, b:b + 1],
                                                    q2red_s, axis=AXX,
                                                    op=ALU.add)
                        emb_ps = pp.tile([P, LC], f32, tag="ps", name="emb_ps")
                        for i, ck in enumerate(range(D2T, C2T)):
                            mm(emb_ps[0:B, :], mv_s[:, ck, :], wo2t_s[:, ck, :],
                               i == 0, False)
                        fin["emb_ps"] = emb_ps

                # ---- stage 2 (ph7 of lc1 sliced into ph8(lc0)'s chain
                # shadows; the prev item's weighted row-sums (tail_ch) spread
                # one per ph7/ph8 block; the next item's DMA+ph1 (or the
                # final q2-column matmuls) slice into ph8(lc1)'s last shadow)
                rrow_ps = [pp.tile([P, LC], f32, tag="ps", name=f"rrow_{b}_{i}")
                           for i in range(NLC)]
                q2T0 = tp.tile([P, D2T, LC], f32r, tag="q2", name=f"q2_{b}_0")
                with ns("ph7.0a"):
                    emit_ph7(hTn_s, xT_s, q2red_s, q2T0, 0, 0, 4)
                if pending:
                    with ns("tail_ch0"):
                        pending[3](0)
                with ns("ph7.0b"):
                    emit_ph7(hTn_s, xT_s, q2red_s, q2T0, 0, 4, D2T)
                if pending:
                    with ns("tail_ch1"):
                        pending[3](1)
                with ns("ph8.0"):
                    pu0 = emit_ph8_nat(hTn_s, xT_s, q2T0, rrow_ps, b, 0)
                q2T1 = tp.tile([P, D2T, LC], f32r, tag="q2", name=f"q2_{b}_1")
                with ns("ph7.1a"):
                    emit_ph7(hTn_s, xT_s, q2red_s, q2T1, 1, 0, 2)
                with ns("pu0"):
                    pu0()
                if b < B - 1:
                    # next item's DMA + ph1 up here: its qT copies must land
                    # in the DVE/ACT FIFOs before the tail pieces' reduces,
                    # or next item's ph2(lc0) stalls on them; the DMA also
                    # gains ~12us of runway
                    with ns("head_dma"):
                        last_fill()
                if pending:
                    with ns("tail_ch2"):
                        pending[3](2)
                with ns("ph7.1b"):
                    emit_ph7(hTn_s, xT_s, q2red_s, q2T1, 1, 2, D2T)
                if pending:
                    with ns("tail_ch3"):
                        pending[3](3)
                    pending = None
                if b < B - 1:
                    with ns("ph1.next0"):
                        head_ph1()

                with ns("ph8.1"):
                    pu1 = emit_ph8_nat(hTn_s, xT_s, q2T1, rrow_ps, b, 1)
                if b == B - 1:
                    last_fill()     # needs the full q2red: must emit late
                with ns("pu1"):
                    pu1()
                if b < B - 1:
                    nxt = nonlocal_state.pop("nxt")

                def fink(ck, last, bb=b):
                    if bb == B - 1:
                        mm(fin["emb_ps"][0:B, :], mv_s[:, ck, :],
                           wo2t_s[:, ck, :], False, last)

                pending = make_tail(b, hTn_s, xT_s, x_s, rrow_ps, q2red_s,
                                    do_q2mv=(b < B - 1),
                                    fink=fink if b == B - 1 else None)

            # last item's tail (final Wo2 matmuls ride inside via fink)
            pending[0]()
            pending[1]()
            pending[2]()
            pending[4]()
            emb_ps = fin["emb_ps"]
            embf_s = vp.tile([B, D], f32, tag="bc", bufs=2, name="embf")
            nc.scalar.copy(embf_s, emb_ps[0:B, :])
            nc.sync.dma_start(out=out_ext[:, :], in_=embf_s)

    _t1 = _time.time()
    nc.compile()
    print(f"[kernel] tile-trace+schedule {_t1 - _t0:.1f}s, "
          f"bacc compile {_time.time() - _t1:.1f}s", file=sys.stderr, flush=True)
    return nc


def get_nc():
    # the pipelined item tail reads xT(b) during item b+1, so the xT pool
    # MUST be double-buffered -- no xt_bufs=1 fallback (it deadlocks)
    if "nc" not in _CACHE:
        _CACHE["nc"] = _build_nc(xt_bufs=2)
    return _CACHE["nc"]


def make_in_maps(x, W1, Wo1, W2, Wo2):
    import ml_dtypes
    x = np.ascontiguousarray(np.asarray(x, dtype=np.float32))
    xT = np.ascontiguousarray(x.transpose(0, 2, 1))
    x_bf = np.ascontiguousarray(x.astype(ml_dtypes.bfloat16))
    x_f8 = np.ascontiguousarray(
        np.clip(x, -240, 240).astype(ml_dtypes.float8_e4m3))
    w1t = np.ascontiguousarray(np.asarray(W1, np.float32).T)
    # Wo1 in fp8 with a x64 power-of-2 scale (values ~0.02 land in e4m3's
    # sweet spot); the ph4 tanh applies scale=1/64
    wo1t8 = np.ascontiguousarray(
        np.clip(np.asarray(Wo1, np.float32).T * 64.0, -240, 240)
        .astype(ml_dtypes.float8_e4m3))
    w2t = np.ascontiguousarray(np.asarray(W2, np.float32).T)
    # 1/L mean-scale folded into Wo2 (it only feeds the final matmuls)
    wo2t = np.ascontiguousarray((np.asarray(Wo2, np.float32).T / L).astype(ml_dtypes.bfloat16))
    onesm = np.ones((P, P), dtype=ml_dtypes.bfloat16)
    onesr = np.ones((1, P), dtype=ml_dtypes.bfloat16)
    return [
        {"x": x_bf[c * B:(c + 1) * B], "x8": x_f8[c * B:(c + 1) * B],
         "xT": xT[c * B:(c + 1) * B],
         "w1t": w1t, "wo1t8": wo1t8, "w2t": w2t, "wo2t": wo2t,
         "onesm": onesm, "onesr": onesr}
        for c in range(NCORES)
    ]


def run(x, W1, Wo1, W2, Wo2, trace=False, **kw):
    from concourse.bass_utils import run_bass_kernel_spmd
    nc = get_nc()
    in_maps = make_in_maps(x, W1, Wo1, W2, Wo2)
    res = run_bass_kernel_spmd(nc, in_maps, core_ids=list(range(NCORES)),
                               trace=trace, **kw)
    out = np.concatenate([res.results[c]["out"] for c in range(NCORES)], axis=0)
    return out.reshape(N_GLOBAL, D, 1, 1), res


def kernel(**inputs):
    out, _ = run(inputs["x"], inputs["W1"], inputs["Wo1"],
                 inputs["W2"], inputs["Wo2"])
    return out


# revision 45
# speedup vs baseline: 1.0038x; 1.0024x over previous
"""AttentionFuserV3 Trainium2 kernel: 8-core pure data parallel over batch.

v2: fp8 DoubleRow on the probability/output matmuls of stage 1.

Reference computation per batch item x_b [L=1024, D=512]:
  stage1: q = x W1^T; S = q x^T; A = softmax(S); mix = A x;
          h = tanh([mix, q] Wo1^T); h = h / max(||h||_2, eps)     (per row)
  stage2: c = [h, x]; q2 = c W2^T; S2 = q2 c^T; A2 = softmax(S2);
          mix2 = A2 c; o = [mix2, q2] Wo2^T; emb = mean_l(o)

Layout: T-space for stage 1 (features on partitions), natural orientation
for stage-2 attention (see ph8).  Stage-2 exploits linearity of the final
mean (r-trick): emb = (1/L)[c^T r ; sum_l q2] Wo2^T.

fp8 (e4m3) with perf_mode=DoubleRow (K=256 per instruction, ~2x PE
throughput at N=512) is used where measured end-to-end error stays small:
  - ph3 (mix = A x): probs are normalized to [0,1] on DVE (exp_bf16 x
    recip -> fp8) and x ships as fp8 from the host.  (3.1e-3 alone)
  - ph4 (out1 = [mix,q] Wo1^T): Wo1 ships as fp8 x64 (tanh applies
    scale=1/64), mix/q are fp8 copies of their psums.  (4.3e-3 alone)
Scores, W1/W2 projections and stage 2 stay f32r: fp8 there measured
1e-2..8e-2 end-to-end (softmax logit noise) -- over the 2e-2 gate.

DVE relief: the stage-2 softmax denominators (dsum) and the q2 column
sums (q2red) ride the ACT accumulator (accum_out) of the exp /
psum-copy instructions instead of DVE tensor_reduces.
"""

import sys

sys.path.insert(0, "/opt/trn_rl_repo")

import numpy as np

N_GLOBAL, L, D = 32, 1024, 512
NCORES = 8
B = N_GLOBAL // NCORES          # 4 batch items per core
P = 128
LC = 512                        # l-chunk (matmul moving free dim)
NLC = L // LC                   # 2
DT = D // P                     # 4
LT = L // P                     # 8
D2T = 2 * D // P                # 8
C2T = 4 * D // P                # 16

_CACHE = {}


def _build_nc(xt_bufs=2):
    import concourse.bass as bass  # noqa: F401
    import concourse.mybir as mybir
    import concourse.tile as tile
    from concourse import bacc

    f32 = mybir.dt.float32
    f32r = mybir.dt.float32r
    bf16 = mybir.dt.bfloat16
    fp8 = mybir.dt.float8e4
    AF = mybir.ActivationFunctionType
    ALU = mybir.AluOpType
    AXX = mybir.AxisListType.X
    DR = mybir.MatmulPerfMode.DoubleRow

    nc = bacc.Bacc("TRN2", target_bir_lowering=False, debug=False,
                   num_devices=NCORES)

    x_ext = nc.declare_dram_parameter("x", [B, L, D], bf16, isOutput=False)
    x8_ext = nc.declare_dram_parameter("x8", [B, L, D], fp8, isOutput=False)
    xT_ext = nc.declare_dram_parameter("xT", [B, D, L], f32r, isOutput=False)
    w1t_ext = nc.declare_dram_parameter("w1t", [D, D], f32r, isOutput=False)
    wo1t8_ext = nc.declare_dram_parameter("wo1t8", [2 * D, D], fp8, isOutput=False)
    w2t_ext = nc.declare_dram_parameter("w2t", [2 * D, 2 * D], f32r, isOutput=False)
    wo2t_ext = nc.declare_dram_parameter("wo2t", [4 * D, D], bf16, isOutput=False)
    # Constants shipped from host: walrus's ISA check rejects memset/iota
    # writes into float32r tiles, but DMA from an f32r DRAM param is fine.
    onm_ext = nc.declare_dram_parameter("onesm", [P, P], bf16, isOutput=False)
    onr_ext = nc.declare_dram_parameter("onesr", [1, P], bf16, isOutput=False)
    out_ext = nc.declare_dram_parameter("out", [B, D], f32, isOutput=True)

    import time as _time
    _t0 = _time.time()
    with tile.TileContext(nc) as tc:
        with tc.tile_pool(name="wp", bufs=1) as wp, \
             tc.tile_pool(name="cp", bufs=1) as cp, \
             tc.tile_pool(name="xtp", bufs=xt_bufs) as xtp, \
             tc.tile_pool(name="xp", bufs=2) as xp, \
             tc.tile_pool(name="hp", bufs=2) as hp, \
             tc.tile_pool(name="tp", bufs=1) as tp, \
             tc.tile_pool(name="vp", bufs=2) as vp, \
             tc.tile_pool(name="ep", bufs=1) as ep, \
             tc.tile_pool(name="ps", bufs=8, space="PSUM") as pp:

            # ---- w1t gates ph1 of item 0: DMA it first (chunks of it
            # interleave with the xT chunks inside the b==0 iteration)
            w1t_s = wp.tile([P, DT, D], f32r, tag="w1t")
            onesm_s = cp.tile([P, P], bf16, tag="onesm")
            nc.sync.dma_start(out=onesm_s, in_=onm_ext[:, :])
            onesr_s = cp.tile([1, P], bf16, tag="onesr")
            nc.sync.dma_start(out=onesr_s, in_=onr_ext[:, :])

            wo1t8_s = wp.tile([P, D2T, D], fp8, tag="wo1t8")
            w2t_s = wp.tile([P, D2T, 2 * D], f32r, tag="w2t")
            wo2t_s = wp.tile([P, C2T, D], bf16, tag="wo2t")

            # meanvec columns for the deferred Wo2 projection:
            # c-chunks 0..3 = sum_l mix2 (h part), 4..7 (x part), 8..15 = sum_l q2
            mv_s = ep.tile([P, C2T, B], bf16, tag="mv")

            def mm(out, lhsT, rhs, first, last, pm=None):
                nc.tensor.matmul(out, lhsT, rhs, start=first, stop=last,
                                 perf_mode=pm)

            def bc_recip(denom_ps, b, lc):
                """[128,512] PSUM pre-broadcast softmax denominator ->
                f32 + bf16 approx reciprocals."""
                bc = vp.tile([P, LC], f32, tag="bc", bufs=2,
                             name=f"bc_{b}_{lc}")
                bcb = vp.tile([P, LC], bf16, tag="bcb", bufs=1,
                              name=f"bcb_{b}_{lc}")
                nc.vector.reciprocal_approx_fast(out=bc, in_=denom_ps)
                with nc.allow_low_precision(reason="bf16 softmax scale"):
                    nc.vector.tensor_copy(bcb, bc)
                return bcb

            def emit_normA8(expT_s, bcb, b, lc):
                """Normalized attention probs in fp8: A8 = expT * (1/denom)."""
                A8_s = tp.tile([P, LT, LC], fp8, tag="a8", bufs=2,
                               name=f"a8_{b}_{lc}")
                with nc.allow_low_precision(reason="fp8 probs"):
                    for mt in range(LT):
                        nc.vector.tensor_mul(A8_s[:, mt, :], expT_s[:, mt, :],
                                             bcb)
                return A8_s

            def emit_ph1(xT_s, b, lc):
                """q chunks: f32r copy (ph2 rhs) + fp8 copy (ph4 rhs)."""
                ls = slice(lc * LC, (lc + 1) * LC)
                qT_s = tp.tile([P, DT, LC], f32r, tag="qt", bufs=1,
                               name=f"qt_{b}_{lc}")
                q8T_s = tp.tile([P, DT, LC], fp8, tag="q8", bufs=2,
                                name=f"q8_{b}_{lc}")
                for et in range(DT):
                    ps = pp.tile([P, LC], f32, tag="ps")
                    for dk in range(DT):
                        mm(ps, w1t_s[:, dk, et * P:(et + 1) * P],
                           xT_s[:, dk, ls], dk == 0, dk == DT - 1)
                    # alternate engines so no single queue backlogs
                    with nc.allow_low_precision(reason="fp8/f32r stores"):
                        if et % 2 == 0:
                            nc.scalar.copy(qT_s[:, et, :], ps)
                            nc.vector.tensor_copy(q8T_s[:, et, :], ps)
                        else:
                            nc.vector.tensor_copy(qT_s[:, et, :], ps)
                            nc.scalar.copy(q8T_s[:, et, :], ps)
                return qT_s, q8T_s

            def emit_ph2(xT_s, qT_s, b, lc):
                expT_s = tp.tile([P, LT, LC], bf16, tag="exp",
                                 name=f"exp1_{b}_{lc}")
                ps_d = pp.tile([P, LC], f32, tag="ps")
                # denominator accumulation deferred one mt so each ones-matmul
                # hides behind the NEXT tile's score matmuls instead of
                # stalling the PE stream on its exp
                for mt in range(LT):
                    ps = pp.tile([P, LC], f32, tag="ps")
                    for ek in range(DT):
                        mm(ps, xT_s[:, ek, mt * P:(mt + 1) * P],
                           qT_s[:, ek, :], ek == 0, ek == DT - 1)
                    if mt > 0:
                        mm(ps_d, onesm_s, expT_s[:, mt - 1, :],
                           mt - 1 == 0, False)
                    nc.scalar.activation(expT_s[:, mt, :], ps, AF.Exp)
                mm(ps_d, onesm_s, expT_s[:, LT - 1, :], False, True)
                return expT_s, ps_d

            def emit_ph3_dr(x8_s, A8_s, b, lc):
                """mix = A x as fp8 DoubleRow over m-chunk pairs; A8 is
                normalized so the psum is the final mix -> fp8 copy."""
                mix8_s = tp.tile([P, DT, LC], fp8, tag="mix8", bufs=1,
                                 name=f"mix8_{b}_{lc}")
                for dt in range(DT):
                    ps = pp.tile([P, LC], f32, tag="ps")
                    for mk in range(0, LT, 2):
                        mm(ps, x8_s[:, mk:mk + 2, dt * P:(dt + 1) * P],
                           A8_s[:, mk:mk + 2, :], mk == 0, mk == LT - 2,
                           pm=DR)
                    with nc.allow_low_precision(reason="fp8 mix store"):
                        nc.scalar.copy(mix8_s[:, dt, :], ps)
                return mix8_s

            def emit_ph4_dr(mix8_s, q8T_s, hTn_s, lc):
                """out1 -> tanh(psum/64) via fp8 DoubleRow over c-chunk
                pairs (Wo1 ships as fp8 x64)."""
                ls = slice(lc * LC, (lc + 1) * LC)
                for ot in range(DT):
                    ps = pp.tile([P, LC], f32, tag="ps")
                    for ck in range(0, D2T, 2):
                        rhs = (mix8_s[:, ck:ck + 2, :] if ck < DT
                               else q8T_s[:, ck - DT:ck - DT + 2, :])
                        mm(ps, wo1t8_s[:, ck:ck + 2, ot * P:(ot + 1) * P],
                           rhs, ck == 0, ck == D2T - 2, pm=DR)
                    nc.scalar.activation(hTn_s[:, ot, ls], ps, AF.Tanh,
                                         scale=1.0 / 64)

            def emit_ph5(hTn_s, b, lc):
                """L2-normalize hTn[:, :, ls] in place (norm over the
                partition axis via ones-matmul)."""
                ls = slice(lc * LC, (lc + 1) * LC)
                # hsq = 16*h^2 in fp8 (the x16 keeps h^2 out of e4m3's
                # subnormal range, where flushing biases the norm low);
                # the sqrt's scale=1/16 undoes it exactly
                hsq_s = tp.tile([P, DT, LC], fp8, tag="hsq", name=f"hsq_{b}_{lc}")
                with nc.allow_low_precision(reason="fp8 16*h^2 for norm"):
                    for dt in range(DT):
                        nc.vector.scalar_tensor_tensor(
                            hsq_s[:, dt, :], hTn_s[:, dt, ls], 16.0,
                            hTn_s[:, dt, ls], op0=ALU.mult, op1=ALU.mult)
                ps_n = pp.tile([P, LC], f32, tag="ps")
                for dt in range(DT):
                    mm(ps_n, onesm_s, hsq_s[:, dt, :], dt == 0, dt == DT - 1)
                bcn = vp.tile([P, LC], f32, tag="bc", bufs=2, name=f"bcn_{b}_{lc}")
                bc2 = vp.tile([P, LC], f32, tag="bc", bufs=2, name=f"bc2_{b}_{lc}")
                nc.scalar.activation(bcn, ps_n, AF.Sqrt, scale=1.0 / 16)
                nc.vector.tensor_scalar_max(bcn, bcn, 1e-12)
                nc.vector.reciprocal_approx_fast(out=bc2, in_=bcn)
                for dt in range(DT):
                    nc.vector.tensor_mul(hTn_s[:, dt, ls], hTn_s[:, dt, ls], bc2)

            def emit_ph7(hTn_s, xT_s, q2red_s, q2T_s, lc, et_lo, et_hi):
                ls = slice(lc * LC, (lc + 1) * LC)

                def c2T(k, fs):
                    return hTn_s[:, k, fs] if k < DT else xT_s[:, k - DT, fs]

                for et in range(et_lo, et_hi):
                    ps = pp.tile([P, LC], f32, tag="ps")
                    for dk in range(D2T):
                        mm(ps, w2t_s[:, dk, et * P:(et + 1) * P],
                           c2T(dk, ls), dk == 0, dk == D2T - 1)
                    # psum->SBUF copy accumulates the q2 column-sum partial
                    # on the ACT accumulator: a DVE tensor_reduce here would
                    # read f32r (2x slow path, ~1.1us) and clog the DVE FIFO
                    # ahead of ph8's u-chain
                    nc.scalar.activation(q2T_s[:, et, :], ps, AF.Copy,
                                         accum_out=q2red_s[:, et, lc:lc + 1])

            def emit_ph8_nat(hTn_s, xT_s, q2T_s, rrow_ps, b, lc):
                """Stage-2 attention in NATURAL orientation (query l on
                partitions): per l-tile, the softmax denominator rides the
                exp's ACT accumulator and r accumulates via matmuls with the
                reciprocal vector as lhsT -- r = sum_lt u_lt^T @ exp2n_lt."""
                def c2T(k, fs):
                    return hTn_s[:, k, fs] if k < DT else xT_s[:, k - DT, fs]

                pend_u = [None]

                def flush_u():
                    if pend_u[0] is not None:
                        pu_b, pe2n, plt = pend_u[0]
                        pend_u[0] = None
                        for ms in range(NLC):
                            mm(rrow_ps[ms][0:1, :], pu_b, pe2n[:, ms, :],
                               plt == 0, plt == LT - 1)

                for li in range(LT // NLC):
                    lt = lc * (LT // NLC) + li
                    loff = li * P
                    e2n_s = tp.tile([P, NLC, LC], bf16, tag="e2n",
                                    name=f"e2n_{b}_{lt}", bufs=2)
                    dsum = vp.tile([P, 3], f32, tag="dsum", bufs=3,
                                   name=f"dsum_{b}_{lt}")
                    ps2 = [pp.tile([P, LC], f32, tag="ps",
                                   name=f"ps8_{b}_{lt}_{i}") for i in range(NLC)]
                    for ek in range(D2T):
                        for ms in range(NLC):
                            mm(ps2[ms], q2T_s[:, ek, loff:loff + P],
                               c2T(ek, slice(ms * LC, (ms + 1) * LC)),
                               ek == 0, ek == D2T - 1)
                    flush_u()   # previous lt's u-matmuls, now chain-covered
                    for ms in range(NLC):
                        nc.scalar.activation(e2n_s[:, ms, :], ps2[ms], AF.Exp,
                                             accum_out=dsum[:, ms:ms + 1])
                    nc.vector.tensor_reduce(dsum[:, 2:3], dsum[:, 0:2],
                                            axis=AXX, op=ALU.add)
                    u_f = vp.tile([P, 1], f32, tag="uf", bufs=3,
                                  name=f"uf_{b}_{lt}")
                    u_b = vp.tile([P, 1], bf16, tag="ub", bufs=3,
                                  name=f"ub_{b}_{lt}")
                    nc.vector.reciprocal_approx_fast(out=u_f, in_=dsum[:, 2:3])
                    with nc.allow_low_precision(reason="bf16 softmax scale"):
                        nc.vector.tensor_copy(u_b, u_f)
                    pend_u[0] = (u_b, e2n_s, lt)
                return flush_u

            def make_tail(b, hTn_s, xT_s, x_s, rrow_ps, q2red_s,
                          do_q2mv=True, fink=None):
                """Item tail, split in three so it can be emitted interleaved
                into the next item's stage-1 engine streams."""
                st = {}

                def tail_a():
                    if do_q2mv:
                        with nc.allow_low_precision(reason="f32r sums"):
                            nc.vector.tensor_reduce(mv_s[:, D2T:C2T, b:b + 1],
                                                    q2red_s, axis=AXX,
                                                    op=ALU.add)
                    rflat_s = vp.tile([1, L], bf16, tag="rflat", bufs=1,
                                      name=f"rflat_{b}")
                    nc.scalar.copy(rflat_s[0:1, 0:LC], rrow_ps[0][0:1, :])
                    nc.scalar.copy(rflat_s[0:1, LC:L], rrow_ps[1][0:1, :])
                    # r row -> column chunks: K=1 matmuls into disjoint
                    # columns of one psum bank
                    rc_ps = pp.tile([P, LT], f32, tag="ps", name=f"rc_{b}")
                    for mt in range(LT):
                        mm(rc_ps[:, mt:mt + 1],
                           rflat_s[0:1, mt * P:(mt + 1) * P],
                           onesr_s[0:1, 0:1], mt == 0, mt == LT - 1)
                    rsum_s = vp.tile([P, LT], bf16, tag="rsum", bufs=1,
                                     name=f"rsum_{b}")
                    with nc.allow_low_precision(reason="bf16 r"):
                        nc.vector.tensor_copy(rsum_s, rc_ps)
                    st["rflat"] = rflat_s
                    st["rsum"] = rsum_s

                def tail_b():
                    rbc_s = vp.tile([P, L], bf16, tag="rbc", bufs=1,
                                    name=f"rbc_{b}")
                    for j in range(NLC):
                        ps_b = pp.tile([P, LC], f32, tag="ps")
                        mm(ps_b, onesr_s, st["rflat"][0:1, j * LC:(j + 1) * LC],
                           True, True)
                        nc.scalar.copy(rbc_s[:, j * LC:(j + 1) * LC], ps_b)
                    st["rbc"] = rbc_s

                def tail_cx():
                    rsum_s = st["rsum"]
                    with nc.allow_low_precision(reason="f32r rounding of sums"):
                        # x part: sum_m x[m,d] r[m] as tiny bf16 matmuls
                        for dt in range(DT):
                            ps_x = pp.tile([P, 1], f32, tag="ps",
                                           name=f"psx_{b}_{dt}")
                            for mk in range(LT):
                                mm(ps_x, x_s[:, mk, dt * P:(dt + 1) * P],
                                   rsum_s[:, mk:mk + 1], mk == 0, mk == LT - 1)
                            nc.vector.tensor_copy(mv_s[:, DT + dt, b:b + 1],
                                                  ps_x)
                            if fink:
                                fink(DT + dt, False)

                def tail_ch(dt):
                    # h part: one weighted row-sum per call -- multiply on
                    # GpSimd (pure slack) into a bf16 scratch, reduce the
                    # bf16 on DVE (f32r reduces hit a 2x slow path and the
                    # in-place f32r write chained Pool behind DVE).
                    # Scratch rides the hsq tag: hsq lives in stage 1, tscr
                    # in stage 2 -- strictly sequential lifetimes.
                    scr = tp.tile([P, L], bf16, tag="hsq", bufs=1,
                                  name=f"tscr_{b}_{dt}")
                    with nc.allow_low_precision(reason="bf16 tail products"):
                        nc.gpsimd.tensor_mul(scr, hTn_s[:, dt, :], st["rbc"])
                        nc.vector.tensor_reduce(mv_s[:, dt, b:b + 1],
                                                scr, axis=AXX, op=ALU.add)
                    if fink:
                        fink(dt, dt == DT - 1)

                def tail_final():
                    # end-of-kernel drain: nothing covers this, so run the
                    # four multiplies two-per-engine first, then the reduces
                    # (+ the final Wo2 matmuls via fink) in order
                    scrs = []
                    # one scratch per dead stage-1 tag so all four products
                    # can be outstanding at once
                    scr_tags = [("hsq", 1), ("exp", 1), ("a8", 2), ("mix8", 1)]
                    with nc.allow_low_precision(reason="bf16 tail products"):
                        for dt in range(DT):
                            tag, tb = scr_tags[dt]
                            scr = tp.tile([P, L], bf16, tag=tag, bufs=tb,
                                          name=f"tscr_{b}_{dt}")
                            eng = nc.gpsimd if dt % 2 == 0 else nc.vector
                            eng.tensor_mul(scr, hTn_s[:, dt, :], st["rbc"])
                            scrs.append(scr)
                        for dt in range(DT):
                            nc.vector.tensor_reduce(mv_s[:, dt, b:b + 1],
                                                    scrs[dt], axis=AXX,
                                                    op=ALU.add)
                            if fink:
                                fink(dt, dt == DT - 1)

                return tail_a, tail_b, tail_cx, tail_ch, tail_final

            pending = None
            nxt = None
            fin = {}
            nonlocal_state = {}
            for b in range(B):
                if nxt is None:
                    xT_s = xtp.tile([P, DT, L], f32r, tag="xT")
                    # per-chunk interleave: ph1's dk=0 matmuls start after
                    # only w1t[0]+xT[0] (~0.75MB) instead of half of both
                    for dk in range(DT):
                        nc.sync.dma_start(out=w1t_s[:, dk:dk + 1, :],
                                          in_=w1t_ext[dk * P:(dk + 1) * P, :]
                                          .rearrange("(k p) e -> p k e", p=P))
                        nc.sync.dma_start(out=xT_s[:, dk:dk + 1, :],
                                          in_=xT_ext[b, dk * P:(dk + 1) * P, :]
                                          .rearrange("(k p) l -> p k l", p=P))
                    x_s = xp.tile([P, LT, D], bf16, tag="x")
                    nc.sync.dma_start(
                        out=x_s, in_=x_ext[b].rearrange("(k p) d -> p k d", p=P))
                    # bufs=1: x8(b)'s last read (ph3dr lc1) lands well before
                    # the next item's prefetch DMA
                    x8_s = xp.tile([P, LT, D], fp8, tag="x8", bufs=1)
                    nc.sync.dma_start(
                        out=x8_s, in_=x8_ext[b].rearrange("(k p) d -> p k d", p=P))
                    nc.sync.dma_start(
                        out=wo1t8_s, in_=wo1t8_ext.rearrange("(k p) e -> p k e", p=P))
                    nc.sync.dma_start(
                        out=w2t_s, in_=w2t_ext.rearrange("(k p) e -> p k e", p=P))
                    nc.sync.dma_start(
                        out=wo2t_s, in_=wo2t_ext.rearrange("(k p) e -> p k e", p=P))
                    qT0, q8T0 = emit_ph1(xT_s, b, 0)
                else:
                    xT_s, x_s, x8_s, qT0, q8T0 = nxt
                    nxt = None
                hTn_s = hp.tile([P, DT, L], f32r, tag="hTn")
                q2red_s = vp.tile([P, D2T, NLC], f32, tag="q2red", bufs=1,
                                  name=f"q2red_{b}")

                # ---- stage 1.  Emission order keeps every DVE chain under
                # PE cover: ph1(lc1)+ph2(lc1) matmuls cover the A8(lc0)
                # normalize chain; ph3/ph4-DR of lc0 + lc1 cover the ph5(lc0)
                # and A8(lc1) chains; ph7(lc0) covers ph5(lc1)+tail_c.
                ns = nc.named_scope
                with ns("ph2.0"):
                    exp0, psd0 = emit_ph2(xT_s, qT0, b, 0)
                with ns("ph1.1"):
                    qT1, q8T1 = emit_ph1(xT_s, b, 1)
                with ns("normA8.0"):
                    bcb0 = bc_recip(psd0, b, 0)
                    A8_0 = emit_normA8(exp0, bcb0, b, 0)
                if pending:
                    with ns("tail_a"):
                        pending[0]()                # r row extraction (PE+ACT)
                    with ns("tail_b"):
                        pending[1]()                # r broadcast (PE+ACT)
                with ns("ph2.1"):
                    exp1, psd1 = emit_ph2(xT_s, qT1, b, 1)
                # A8(lc1) chain ahead of ph5(lc0)'s in the DVE FIFO: it only
                # waits on psd1 and must not queue behind the norm chain
                # (ph3dr(lc1) would stall on it)
                with ns("normA8.1"):
                    bcb1 = bc_recip(psd1, b, 1)
                    A8_1 = emit_normA8(exp1, bcb1, b, 1)
                with ns("ph3dr.0"):
                    mix8_0 = emit_ph3_dr(x8_s, A8_0, b, 0)
                with ns("ph4dr.0"):
                    emit_ph4_dr(mix8_0, q8T0, hTn_s, 0)
                with ns("ph5.0"):
                    emit_ph5(hTn_s, b, 0)
                with ns("ph3dr.1"):
                    mix8_1 = emit_ph3_dr(x8_s, A8_1, b, 1)
                with ns("ph4dr.1"):
                    emit_ph4_dr(mix8_1, q8T1, hTn_s, 1)
                with ns("ph5.1"):
                    emit_ph5(hTn_s, b, 1)
                if pending:
                    with ns("tail_cx"):
                        pending[2]()                # x-part row-sums (PE+DVE)

                if b < B - 1:
                    def head_dma(bn=b + 1):
                        xTn = xtp.tile([P, DT, L], f32r, tag="xT")
                        nc.sync.dma_start(
                            out=xTn[:, 0:2, :],
                            in_=xT_ext[bn, 0:2 * P, :]
                            .rearrange("(k p) l -> p k l", p=P))
                        nc.sync.dma_start(
                            out=xTn[:, 2:DT, :],
                            in_=xT_ext[bn, 2 * P:DT * P, :]
                            .rearrange("(k p) l -> p k l", p=P))
                        xn = xp.tile([P, LT, D], bf16, tag="x")
                        nc.sync.dma_start(
                            out=xn,
                            in_=x_ext[bn].rearrange("(k p) d -> p k d", p=P))
                        x8n = xp.tile([P, LT, D], fp8, tag="x8", bufs=1)
                        nc.sync.dma_start(
                            out=x8n,
                            in_=x8_ext[bn].rearrange("(k p) d -> p k d", p=P))
                        nonlocal_state["dma"] = (xTn, xn, x8n)

                    def head_ph1(bn=b + 1):
                        xTn, xn, x8n = nonlocal_state.pop("dma")
                        qn, q8n = emit_ph1(xTn, bn, 0)
                        nonlocal_state["nxt"] = (xTn, xn, x8n, qn, q8n)
                    last_fill = head_dma
                else:
                    def last_fill():
                        with nc.allow_low_precision(reason="f32r sums"):
                            nc.vector.tensor_reduce(mv_s[:, D2T:C2T, b:b + 1],
                                                    q2red_s, axis=AXX,
                                                    op=ALU.add)
                        emb_ps = pp.tile([P, LC], f32, tag="ps", name="emb_ps")
                        for i, ck in enumerate(range(D2T, C2T)):
                            mm(emb_ps[0:B, :], mv_s[:, ck, :], wo2t_s[:, ck, :],
                               i == 0, False)
                        fin["emb_ps"] = emb_ps

                # ---- stage 2 (ph7 of lc1 sliced into ph8(lc0)'s chain
                # shadows; the prev item's weighted row-sums (tail_ch) spread
                # one per ph7/ph8 block; the next item's DMA+ph1 (or the
                # final q2-column matmuls) slice into ph8(lc1)'s last shadow)
                rrow_ps = [pp.tile([P, LC], f32, tag="ps", name=f"rrow_{b}_{i}")
                           for i in range(NLC)]
                q2T0 = tp.tile([P, D2T, LC], f32r, tag="q2", name=f"q2_{b}_0")
                with ns("ph7.0a"):
                    emit_ph7(hTn_s, xT_s, q2red_s, q2T0, 0, 0, 4)
                if pending:
                    with ns("tail_ch0"):
                        pending[3](0)
                with ns("ph7.0b"):
                    emit_ph7(hTn_s, xT_s, q2red_s, q2T0, 0, 4, D2T)
                if pending:
                    with ns("tail_ch1"):
                        pending[3](1)
                with ns("ph8.0"):
                    pu0 = emit_ph8_nat(hTn_s, xT_s, q2T0, rrow_ps, b, 0)
                q2T1 = tp.tile([P, D2T, LC], f32r, tag="q2", name=f"q2_{b}_1")
                with ns("ph7.1a"):
                    emit_ph7(hTn_s, xT_s, q2red_s, q2T1, 1, 0, 2)
                with ns("pu0"):
                    pu0()
                if b < B - 1:
                    # next item's DMA + ph1 up here: its qT copies must land
                    # in the DVE/ACT FIFOs before the tail pieces' reduces,
                    # or next item's ph2(lc0) stalls on them; the DMA also
                    # gains ~12us of runway
                    with ns("head_dma"):
                        last_fill()
                if pending:
                    with ns("tail_ch2"):
                        pending[3](2)
                with ns("ph7.1b"):
                    emit_ph7(hTn_s, xT_s, q2red_s, q2T1, 1, 2, D2T)
                if pending:
                    with ns("tail_ch3"):
                        pending[3](3)
                    pending = None
                if b < B - 1:
                    with ns("ph1.next0"):
                        head_ph1()

                with ns("ph8.1"):
                    pu1 = emit_ph8_nat(hTn_s, xT_s, q2T1, rrow_ps, b, 1)
                if b == B - 1:
                    last_fill()     # needs the full q2red: must emit late
                with ns("pu1"):
                    pu1()
                if b < B - 1:
                    nxt = nonlocal_state.pop("nxt")

                def fink(ck, last, bb=b):
                    if bb == B - 1:
                        mm(fin["emb_ps"][0:B, :], mv_s[:, ck, :],
                           wo2t_s[:, ck, :], False, last)

                pending = make_tail(b, hTn_s, xT_s, x_s, rrow_ps, q2red_s,
                                    do_q2mv=(b < B - 1),
                                    fink=fink if b == B - 1 else None)

            # last item's tail (final Wo2 matmuls ride inside via fink)
            pending[0]()
            pending[1]()
            pending[2]()
            pending[4]()
            emb_ps = fin["emb_ps"]
            embf_s = vp.tile([B, D], f32, tag="bc", bufs=2, name="embf")
            nc.scalar.copy(embf_s, emb_ps[0:B, :])
            nc.sync.dma_start(out=out_ext[:, :], in_=embf_s)

    _t1 = _time.time()
    nc.compile()
    print(f"[kernel] tile-trace+schedule {_t1 - _t0:.1f}s, "
          f"bacc compile {_time.time() - _t1:.1f}s", file=sys.stderr, flush=True)
    return nc


def get_nc():
    # the pipelined item tail reads xT(b) during item b+1, so the xT pool
    # MUST be double-buffered -- no xt_bufs=1 fallback (it deadlocks)
    if "nc" not in _CACHE:
        _CACHE["nc"] = _build_nc(xt_bufs=2)
    return _CACHE["nc"]


def make_in_maps(x, W1, Wo1, W2, Wo2):
    import ml_dtypes
    x = np.ascontiguousarray(np.asarray(x, dtype=np.float32))
    xT = np.ascontiguousarray(x.transpose(0, 2, 1))
    x_bf = np.ascontiguousarray(x.astype(ml_dtypes.bfloat16))
    x_f8 = np.ascontiguousarray(
        np.clip(x, -240, 240).astype(ml_dtypes.float8_e4m3))
    w1t = np.ascontiguousarray(np.asarray(W1, np.float32).T)
    # Wo1 in fp8 with a x64 power-of-2 scale (values ~0.02 land in e4m3's
    # sweet spot); the ph4 tanh applies scale=1/64
    wo1t8 = np.ascontiguousarray(
        np.clip(np.asarray(Wo1, np.float32).T * 64.0, -240, 240)
        .astype(ml_dtypes.float8_e4m3))
    w2t = np.ascontiguousarray(np.asarray(W2, np.float32).T)
    # 1/L mean-scale folded into Wo2 (it only feeds the final matmuls)
    wo2t = np.ascontiguousarray((np.asarray(Wo2, np.float32).T / L).astype(ml_dtypes.bfloat16))
    onesm = np.ones((P, P), dtype=ml_dtypes.bfloat16)
    onesr = np.ones((1, P), dtype=ml_dtypes.bfloat16)
    return [
        {"x": x_bf[c * B:(c + 1) * B], "x8": x_f8[c * B:(c + 1) * B],
         "xT": xT[c * B:(c + 1) * B],
         "w1t": w1t, "wo1t8": wo1t8, "w2t": w2t, "wo2t": wo2t,
         "onesm": onesm, "onesr": onesr}
        for c in range(NCORES)
    ]


def run(x, W1, Wo1, W2, Wo2, trace=False, **kw):
    from concourse.bass_utils import run_bass_kernel_spmd
    nc = get_nc()
    in_maps = make_in_maps(x, W1, Wo1, W2, Wo2)
    res = run_bass_kernel_spmd(nc, in_maps, core_ids=list(range(NCORES)),
                               trace=trace, **kw)
    out = np.concatenate([res.results[c]["out"] for c in range(NCORES)], axis=0)
    return out.reshape(N_GLOBAL, D, 1, 1), res


def kernel(**inputs):
    out, _ = run(inputs["x"], inputs["W1"], inputs["Wo1"],
                 inputs["W2"], inputs["Wo2"])
    return out
